# revision 12
# baseline (speedup 1.0000x reference)
"""MQA attention kernel (B=2, T=2048, C=2048, 16 query heads, D=128, RoPE,
causal) for 8 Trainium2 NeuronCores.

Sharding: core = (batch, head-group-of-4). Each core computes q projection for
its 4 heads, the full shared K/V projection for its batch (MQA), causal
attention, and a partial output projection; the host sums the 4 partials per
batch.

Device layout notes:
  - Host pre-transposes x to xT [C, T] so all contractions over C read
    contiguous DRAM.
  - RoPE's even/odd interleave is turned into a half-split layout by permuting
    Wq/Wk columns on the host (scores are invariant to a shared permutation of
    the head dim).  Wq is also pre-scaled by 1/sqrt(D).
  - Scores are computed transposed, S.T[j, i], so the p@V and output
    projections need no on-chip transposes; softmax denominators come from a
    ones-row matmul and are applied via a DRAM-broadcast + reciprocal.
  - All matmuls run as float32r (full PE rate for moving dim >= 256).
"""

import os
import sys

if "/opt/trn_rl_repo" not in sys.path:
    sys.path.insert(0, "/opt/trn_rl_repo")

import numpy as np

import concourse.bacc as bacc
import concourse.mybir as mybir
import concourse.tile as tile
from concourse.bass_utils import run_bass_kernel_spmd

T = 2048
C = 2048
D = 128
N_HEAD = 16
HPC = 4  # heads per core
N_CORES = 8
F32 = mybir.dt.float32
F32R = mybir.dt.float32r
EXP = mybir.ActivationFunctionType.Exp


KPHASE = int(os.environ.get("KPHASE", "9"))
KSUM = int(os.environ.get("KSUM", "1"))
KEXP = int(os.environ.get("KEXP", "1"))


def build_program():
    nc = bacc.Bacc("TRN2", target_bir_lowering=False, debug=False)

    xt = nc.dram_tensor("xt", [C, T], F32R, kind="ExternalInput")
    wq = nc.dram_tensor("wq", [C, HPC * D], F32R, kind="ExternalInput")
    wk = nc.dram_tensor("wk", [C, D], F32R, kind="ExternalInput")
    wv = nc.dram_tensor("wv", [C, D], F32R, kind="ExternalInput")
    wo = nc.dram_tensor("wo", [HPC * D, C], F32R, kind="ExternalInput")
    cc = nc.dram_tensor("cc", [D, T], F32, kind="ExternalInput")
    ss = nc.dram_tensor("ss", [D, T], F32, kind="ExternalInput")
    ones_d = nc.dram_tensor("ones_d", [128, 128], F32R, kind="ExternalInput")
    ident_d = nc.dram_tensor("ident_d", [128, 128], F32R, kind="ExternalInput")
    out = nc.dram_tensor("out", [T, C], F32, kind="ExternalOutput")

    xt_r = xt.rearrange("(ko p) t -> p ko t", p=128)
    wq_r = wq.rearrange("(ko p) m -> p ko m", p=128)
    wk_r = wk.rearrange("(ko p) m -> p ko m", p=128)
    wv_r = wv.rearrange("(ko p) m -> p ko m", p=128)
    wo_r = wo.rearrange("(ho p) c -> p ho c", p=128)
    out_r = out.rearrange("(mo p) c -> p mo c", p=128)

    with (
        tile.TileContext(nc) as tc,
        tc.tile_pool(name="consts", bufs=1) as consts,
        tc.tile_pool(name="qkpool", bufs=5) as qkpool,
        tc.tile_pool(name="ytpool", bufs=4) as ytpool,
        tc.tile_pool(name="wpool", bufs=1) as wpool,
    ):
        ident = consts.tile([128, 128], F32R, tag="ident")
        nc.sync.dma_start(out=ident, in_=ident_d[:, :])
        ones = consts.tile([128, 128], F32R, tag="ones")
        nc.sync.dma_start(out=ones, in_=ones_d[:, :])
        # tri[j, i] = 1 if i >= j else 0 (keep causal-valid entries)
        tri = consts.tile([128, 128], F32, tag="tri")
        nc.gpsimd.memset(tri, 1.0)
        nc.gpsimd.affine_select(
            out=tri,
            in_=tri,
            compare_op=mybir.AluOpType.is_ge,
            fill=0.0,
            base=0,
            pattern=[[1, 128]],
            channel_multiplier=-1,
        )
        ccs = consts.tile([128, T], F32, tag="cc")
        nc.sync.dma_start(out=ccs, in_=cc[:, :])
        sss = consts.tile([128, T], F32, tag="ss")
        nc.sync.dma_start(out=sss, in_=ss[:, :])
        wks = consts.tile([128, 16, 128], F32R, tag="wk")
        nc.sync.dma_start(out=wks, in_=wk_r)
        wvs = consts.tile([128, 16, 128], F32R, tag="wv")
        nc.sync.dma_start(out=wvs, in_=wv_r)
        vsb = consts.tile([128, 16, 128], F32R, tag="vsb")  # v, natural [t, d]

        wqs = wpool.tile([128, 16, 512], F32R, tag="w")
        nc.sync.dma_start(out=wqs, in_=wq_r)

        # qk[0..3] = q.T per head, qk[4] = k.T -- [d, t] layout, RoPE'd in place
        qk = [qkpool.tile([128, T], F32R, tag="qk", name=f"qk{i}") for i in range(5)]
        yt = [ytpool.tile([128, T], F32R, tag="yt", name=f"yt{i}") for i in range(4)]

        # ---- phase 1: q/k/v projections (contraction over C) ----
        with (
            tc.tile_pool(name="xtp", bufs=3) as xtp,
            tc.tile_pool(name="projps", bufs=8, space="PSUM") as projps,
            tc.tile_pool(name="b2k", bufs=3) as b2k,
        ):
            vtt = b2k.tile([128, T], F32R, tag="b2k")  # v.T staging
            for tcn in range(4):
                tsl = slice(tcn * 512, (tcn + 1) * 512)
                pq = [projps.tile([128, 512], F32, tag="ps", name=f"pq{i}") for i in range(4)]
                pk = projps.tile([128, 512], F32, tag="ps")
                pv = projps.tile([128, 512], F32, tag="ps")
                for k in range(16):
                    xtt = xtp.tile([128, 512], F32R, tag="xt")
                    nc.sync.dma_start(out=xtt, in_=xt_r[:, k, tsl])
                    st, sp = k == 0, k == 15
                    for h in range(4):
                        nc.tensor.matmul(
                            pq[h],
                            (wqs[:, k, h * 128 : (h + 1) * 128]),
                            (xtt),
                            start=st,
                            stop=sp,
                        )
                    nc.tensor.matmul(pk, (wks[:, k, :]), (xtt), start=st, stop=sp)
                    nc.tensor.matmul(pv, (wvs[:, k, :]), (xtt), start=st, stop=sp)
                for h in range(4):
                    nc.scalar.copy(out=qk[h][:, tsl], in_=pq[h])
                nc.scalar.copy(out=qk[4][:, tsl], in_=pk)
                nc.scalar.copy(out=vtt[:, tsl], in_=pv)

            # v.T -> v (natural [t, d]) via PE transpose
            for m in range(16):
                pt = projps.tile([128, 512], F32R, tag="ps")
                nc.tensor.transpose(pt[:, :128], vtt[:, m * 128 : (m + 1) * 128], ident)
                nc.vector.tensor_copy(out=vsb[:, m, :], in_=pt[:, :128])

            # RoPE in the half-split layout: rot = q*CC + swap(q)*SS
            for idx in range(5):
                sw = b2k.tile([128, T], F32R, tag="b2k")
                nc.gpsimd.dma_start(out=sw[0:64, :], in_=qk[idx][64:128, :])
                nc.gpsimd.dma_start(out=sw[64:128, :], in_=qk[idx][0:64, :])
                nc.vector.tensor_mul(out=qk[idx][:], in0=qk[idx][:], in1=ccs[:])
                nc.vector.tensor_mul(out=sw[:], in0=sw[:], in1=sss[:])
                nc.vector.tensor_add(out=qk[idx][:], in0=qk[idx][:], in1=sw[:])

        # ---- phase 2: causal attention, scores transposed S.T[j, i] ----
        with (
            tc.tile_pool(name="t512", bufs=8) as t512,
            tc.tile_pool(name="psA", bufs=4, space="PSUM") as psA,
            tc.tile_pool(name="psY", bufs=2, space="PSUM") as psY,
            tc.tile_pool(name="psS", bufs=2, space="PSUM") as psS,
        ):
            for h in range(4 if KPHASE >= 2 else 0):
                for c in range(4):
                    i0 = c * 512
                    py = psY.tile([128, 512], F32, tag="py")
                    psm = psS.tile([128, 512], F32, tag="pss")
                    njj = 4 * c + 4
                    for jj in range(njj):
                        r = jj - 4 * c  # >= 0 only for diagonal-group tiles
                        off = 128 * r if r >= 0 else 0
                        pss = psA.tile([128, 512], F32, tag="ps")
                        nc.tensor.matmul(
                            pss[:, off:],
                            (qk[4][:, jj * 128 : (jj + 1) * 128]),
                            (qk[h][:, i0 + off : i0 + 512]),
                            start=True,
                            stop=True,
                        )
                        pT = t512.tile([128, 512], F32R, tag="t512")
                        if KEXP:
                            nc.scalar.activation(out=pT[:, off:], in_=pss[:, off:], func=EXP)
                        else:
                            nc.scalar.copy(out=pT[:, off:], in_=pss[:, off:])
                        if r >= 0:
                            nc.vector.tensor_mul(
                                out=pT[:, off : off + 128],
                                in0=pT[:, off : off + 128],
                                in1=tri,
                            )
                        st, sp = jj == 0, jj == njj - 1
                        nc.tensor.matmul(
                            py[:, off:],
                            (vsb[:, jj, :]),
                            (pT[:, off:]),
                            start=st,
                            stop=sp,
                        )
                        nc.tensor.matmul(
                            psm[:, off:],
                            (ones),
                            (pT[:, off:]),
                            start=st,
                            stop=sp,
                        )
                    bc = t512.tile([128, 512], F32, tag="t512")
                    nc.vector.reciprocal_approx_fast(out=bc, in_=psm)
                    nc.vector.tensor_mul(
                        out=yt[h][:, i0 : i0 + 512], in0=py, in1=bc
                    )

            # ---- phase 3: partial output projection (contraction over d) ----
            wos = wpool.tile([128, 4, T], F32R, tag="w")
            nc.sync.dma_start(out=wos, in_=wo_r)
            for m in range(16 if KPHASE >= 4 else 0):
                for cn in range(4):
                    po = psA.tile([128, 512], F32, tag="ps")
                    for h in range(4):
                        nc.tensor.matmul(
                            po,
                            (yt[h][:, m * 128 : (m + 1) * 128]),
                            (wos[:, h, cn * 512 : (cn + 1) * 512]),
                            start=h == 0,
                            stop=h == 3,
                        )
                    ot = t512.tile([128, 512], F32, tag="t512")
                    nc.any.tensor_copy(out=ot, in_=po)
                    nc.sync.dma_start(out=out_r[:, m, cn * 512 : (cn + 1) * 512], in_=ot)

    nc.compile()
    return nc


_PERM = np.concatenate([np.arange(0, D, 2), np.arange(1, D, 2)])


def make_in_maps(x, freqs_cos, freqs_sin, Wq, Wk, Wv, Wo):
    x = np.asarray(x, dtype=np.float32)
    freqs_cos = np.asarray(freqs_cos, dtype=np.float32)
    freqs_sin = np.asarray(freqs_sin, dtype=np.float32)
    Wq = np.asarray(Wq, dtype=np.float32)
    Wk = np.asarray(Wk, dtype=np.float32)
    Wv = np.asarray(Wv, dtype=np.float32)
    Wo = np.asarray(Wo, dtype=np.float32)

    scale = 1.0 / np.sqrt(np.float32(D))
    cosT = np.ascontiguousarray(freqs_cos.T)  # [64, T]
    sinT = np.ascontiguousarray(freqs_sin.T)
    cc = np.ascontiguousarray(np.concatenate([cosT, cosT], axis=0))  # [128, T]
    ss = np.ascontiguousarray(np.concatenate([-sinT, sinT], axis=0))
    wk_p = np.ascontiguousarray(Wk[:, _PERM])
    wv_c = np.ascontiguousarray(Wv)

    xts = [np.ascontiguousarray(x[b].T) for b in range(2)]

    ones_a = np.ones((128, 128), dtype=np.float32)
    ident_a = np.eye(128, dtype=np.float32)
    in_maps = []
    for core in range(N_CORES):
        b = core // 4
        hg = core % 4
        heads = range(4 * hg, 4 * hg + 4)
        qcols = np.concatenate([h * D + _PERM for h in heads])
        wq_c = np.ascontiguousarray(Wq[:, qcols] * scale)
        orows = np.concatenate([np.arange(h * D, (h + 1) * D) for h in heads])
        wo_c = np.ascontiguousarray(Wo[orows, :])
        in_maps.append(
            {
                "xt": xts[b],
                "wq": wq_c,
                "wk": wk_p,
                "wv": wv_c,
                "wo": wo_c,
                "cc": cc,
                "ss": ss,
                "ones_d": ones_a,
                "ident_d": ident_a,
            }
        )
    return in_maps


_PROGRAM = None


def get_program():
    global _PROGRAM
    if _PROGRAM is None:
        _PROGRAM = build_program()
    return _PROGRAM


def kernel(x, freqs_cos, freqs_sin, Wq, Wk, Wv, Wo, _collect=None):
    nc = get_program()
    in_maps = make_in_maps(x, freqs_cos, freqs_sin, Wq, Wk, Wv, Wo)
    res = run_bass_kernel_spmd(nc, in_maps, core_ids=list(range(N_CORES)))
    if _collect is not None:
        _collect.append(res)
    outs = [r["out"] for r in res.results]
    full = np.empty((2, T, C), dtype=np.float32)
    for b in range(2):
        full[b] = outs[4 * b] + outs[4 * b + 1] + outs[4 * b + 2] + outs[4 * b + 3]
    return full


# revision 14
# speedup vs baseline: 1.2169x; 1.2169x over previous
"""MQA attention kernel (B=2, T=2048, C=2048, 16 query heads, D=128, RoPE,
causal) for 8 Trainium2 NeuronCores.

Sharding: core = (batch, head-group-of-4). Each core computes q projection for
its 4 heads, the full shared K/V projection for its batch (MQA), causal
attention, and a partial output projection; the host sums the 4 partials per
batch.

Device layout notes:
  - Host pre-transposes x to xT [C, T] so all contractions over C read
    contiguous DRAM.
  - RoPE's even/odd interleave is turned into a half-split layout by permuting
    Wq/Wk columns on the host (scores are invariant to a shared permutation of
    the head dim).  Wq is also pre-scaled by 1/sqrt(D).
  - Scores are computed transposed, S.T[j, i], so the p@V and output
    projections need no on-chip transposes; softmax denominators come from a
    ones-row matmul and are applied via a DRAM-broadcast + reciprocal.
  - All matmuls run as float32r (full PE rate for moving dim >= 256).
"""

import os
import sys

if "/opt/trn_rl_repo" not in sys.path:
    sys.path.insert(0, "/opt/trn_rl_repo")

import numpy as np

import concourse.bacc as bacc
import concourse.mybir as mybir
import concourse.tile as tile
from concourse.bass_utils import run_bass_kernel_spmd

T = 2048
C = 2048
D = 128
N_HEAD = 16
HPC = 4  # heads per core
N_CORES = 8
F32 = mybir.dt.float32
F32R = mybir.dt.float32r
EXP = mybir.ActivationFunctionType.Exp


KPHASE = int(os.environ.get("KPHASE", "9"))
KSUM = int(os.environ.get("KSUM", "1"))
KEXP = int(os.environ.get("KEXP", "1"))


def build_program():
    nc = bacc.Bacc("TRN2", target_bir_lowering=False, debug=False)

    xt = nc.dram_tensor("xt", [C, T], F32R, kind="ExternalInput")
    wq = nc.dram_tensor("wq", [C, HPC * D], F32R, kind="ExternalInput")
    wk = nc.dram_tensor("wk", [C, D], F32R, kind="ExternalInput")
    wv = nc.dram_tensor("wv", [C, D], F32R, kind="ExternalInput")
    wo = nc.dram_tensor("wo", [HPC * D, C], F32R, kind="ExternalInput")
    cc = nc.dram_tensor("cc", [D, T], F32, kind="ExternalInput")
    ss = nc.dram_tensor("ss", [D, T], F32, kind="ExternalInput")
    ones_d = nc.dram_tensor("ones_d", [128, 128], F32R, kind="ExternalInput")
    ident_d = nc.dram_tensor("ident_d", [128, 128], F32R, kind="ExternalInput")
    out = nc.dram_tensor("out", [T, C], F32, kind="ExternalOutput")

    xt_r = xt.rearrange("(ko p) t -> p ko t", p=128)
    wq_r = wq.rearrange("(ko p) m -> p ko m", p=128)
    wk_r = wk.rearrange("(ko p) m -> p ko m", p=128)
    wv_r = wv.rearrange("(ko p) m -> p ko m", p=128)
    wo_r = wo.rearrange("(ho p) c -> p ho c", p=128)
    out_r = out.rearrange("(mo p) c -> p mo c", p=128)

    with (
        tile.TileContext(nc) as tc,
        tc.tile_pool(name="consts", bufs=1) as consts,
        tc.tile_pool(name="qkpool", bufs=5) as qkpool,
        tc.tile_pool(name="ytpool", bufs=4) as ytpool,
        tc.tile_pool(name="wpool", bufs=1) as wpool,
    ):
        ident = consts.tile([128, 128], F32R, tag="ident")
        nc.sync.dma_start(out=ident, in_=ident_d[:, :])
        ones = consts.tile([128, 128], F32R, tag="ones")
        nc.sync.dma_start(out=ones, in_=ones_d[:, :])
        # tri[j, i] = 1 if i >= j else 0 (keep causal-valid entries)
        tri = consts.tile([128, 128], F32, tag="tri")
        nc.gpsimd.memset(tri, 1.0)
        nc.gpsimd.affine_select(
            out=tri,
            in_=tri,
            compare_op=mybir.AluOpType.is_ge,
            fill=0.0,
            base=0,
            pattern=[[1, 128]],
            channel_multiplier=-1,
        )
        ccs = consts.tile([128, T], F32, tag="cc")
        nc.sync.dma_start(out=ccs, in_=cc[:, :])
        sss = consts.tile([128, T], F32, tag="ss")
        nc.sync.dma_start(out=sss, in_=ss[:, :])
        wks = consts.tile([128, 16, 128], F32R, tag="wk")
        nc.sync.dma_start(out=wks, in_=wk_r)
        wvs = consts.tile([128, 16, 128], F32R, tag="wv")
        nc.sync.dma_start(out=wvs, in_=wv_r)
        vsb = consts.tile([128, 16, 128], F32R, tag="vsb")  # v, natural [t, d]

        wqs = wpool.tile([128, 16, 512], F32R, tag="w")
        nc.sync.dma_start(out=wqs, in_=wq_r)

        # qk[0..3] = q.T per head, qk[4] = k.T -- [d, t] layout, RoPE'd in place
        qk = [qkpool.tile([128, T], F32R, tag="qk", name=f"qk{i}") for i in range(5)]
        yt = [ytpool.tile([128, T], F32R, tag="yt", name=f"yt{i}") for i in range(4)]

        # ---- phase 1: q/k/v projections (contraction over C) ----
        with (
            tc.tile_pool(name="t512", bufs=18) as t512,
            tc.tile_pool(name="psA", bufs=4, space="PSUM") as psA,
            tc.tile_pool(name="psY", bufs=2, space="PSUM") as psY,
            tc.tile_pool(name="psS", bufs=2, space="PSUM") as psS,
        ):
            for tcn in range(4):
                tsl = slice(tcn * 512, (tcn + 1) * 512)
                xts = []
                for k in range(16):
                    xtt = t512.tile([128, 512], F32R, tag="t512", name=f"xt{tcn}_{k}")
                    nc.sync.dma_start(out=xtt, in_=xt_r[:, k, tsl])
                    xts.append(xtt)
                # group 1: the 4 q heads
                pq = [psA.tile([128, 512], F32, tag="ps", name=f"pq{i}") for i in range(4)]
                for k in range(16):
                    st, sp = k == 0, k == 15
                    for h in range(4):
                        nc.tensor.matmul(
                            pq[h],
                            wqs[:, k, h * 128 : (h + 1) * 128],
                            xts[k],
                            start=st,
                            stop=sp,
                        )
                # group 2: kT and vT
                pk = psA.tile([128, 512], F32, tag="ps")
                pv = psA.tile([128, 512], F32, tag="ps")
                for k in range(16):
                    st, sp = k == 0, k == 15
                    nc.tensor.matmul(pk, wks[:, k, :], xts[k], start=st, stop=sp)
                    nc.tensor.matmul(pv, wvs[:, k, :], xts[k], start=st, stop=sp)
                for h in range(4):
                    nc.scalar.copy(out=qk[h][:, tsl], in_=pq[h])
                nc.scalar.copy(out=qk[4][:, tsl], in_=pk)

                # v natural tiles for this chunk via PE transpose
                vtt = t512.tile([128, 512], F32R, tag="t512", name=f"vtt{tcn}")
                nc.scalar.copy(out=vtt, in_=pv)
                for mm in range(4):
                    m = tcn * 4 + mm
                    ptp = psA.tile([128, 512], F32R, tag="ps", name=f"ptp{m}")
                    nc.tensor.transpose(
                        ptp[:, :128], vtt[:, mm * 128 : (mm + 1) * 128], ident
                    )
                    nc.vector.tensor_copy(out=vsb[:, m, :], in_=ptp[:, :128])

                # RoPE this chunk (k first so attention unblocks earliest)
                for idx in [4, 0, 1, 2, 3]:
                    sw = t512.tile([128, 512], F32R, tag="t512", name=f"sw{tcn}_{idx}")
                    nc.gpsimd.dma_start(out=sw[0:64, :], in_=qk[idx][64:128, tsl])
                    nc.gpsimd.dma_start(out=sw[64:128, :], in_=qk[idx][0:64, tsl])
                    nc.vector.tensor_mul(
                        out=qk[idx][:, tsl], in0=qk[idx][:, tsl], in1=ccs[:, tsl]
                    )
                    nc.vector.tensor_mul(out=sw[:], in0=sw[:], in1=sss[:, tsl])
                    nc.vector.tensor_add(
                        out=qk[idx][:, tsl], in0=qk[idx][:, tsl], in1=sw[:]
                    )

            # ---- phase 2: causal attention, scores transposed S.T[j, i] ----
            for h in range(4 if KPHASE >= 2 else 0):
                for c in range(4):
                    i0 = c * 512
                    py = psY.tile([128, 512], F32, tag="py")
                    psm = psS.tile([128, 512], F32, tag="pss")
                    njj = 4 * c + 4
                    for jj in range(njj):
                        r = jj - 4 * c  # >= 0 only for diagonal-group tiles
                        off = 128 * r if r >= 0 else 0
                        pss = psA.tile([128, 512], F32, tag="ps")
                        nc.tensor.matmul(
                            pss[:, off:],
                            (qk[4][:, jj * 128 : (jj + 1) * 128]),
                            (qk[h][:, i0 + off : i0 + 512]),
                            start=True,
                            stop=True,
                        )
                        pT = t512.tile([128, 512], F32R, tag="t512")
                        if KEXP:
                            nc.scalar.activation(out=pT[:, off:], in_=pss[:, off:], func=EXP)
                        else:
                            nc.scalar.copy(out=pT[:, off:], in_=pss[:, off:])
                        if r >= 0:
                            nc.vector.tensor_mul(
                                out=pT[:, off : off + 128],
                                in0=pT[:, off : off + 128],
                                in1=tri,
                            )
                        st, sp = jj == 0, jj == njj - 1
                        nc.tensor.matmul(
                            py[:, off:],
                            (vsb[:, jj, :]),
                            (pT[:, off:]),
                            start=st,
                            stop=sp,
                        )
                        nc.tensor.matmul(
                            psm[:, off:],
                            (ones),
                            (pT[:, off:]),
                            start=st,
                            stop=sp,
                        )
                    bc = t512.tile([128, 512], F32, tag="t512")
                    nc.vector.reciprocal_approx_fast(out=bc, in_=psm)
                    nc.vector.tensor_mul(
                        out=yt[h][:, i0 : i0 + 512], in0=py, in1=bc
                    )

            # ---- phase 3: partial output projection (contraction over d) ----
            wos = wpool.tile([128, 4, T], F32R, tag="w")
            nc.gpsimd.dma_start(out=wos, in_=wo_r)
            for m in range(16 if KPHASE >= 4 else 0):
                for cn in range(4):
                    po = psA.tile([128, 512], F32, tag="ps")
                    for h in range(4):
                        nc.tensor.matmul(
                            po,
                            (yt[h][:, m * 128 : (m + 1) * 128]),
                            (wos[:, h, cn * 512 : (cn + 1) * 512]),
                            start=h == 0,
                            stop=h == 3,
                        )
                    ot = t512.tile([128, 512], F32, tag="t512")
                    nc.any.tensor_copy(out=ot, in_=po)
                    nc.sync.dma_start(out=out_r[:, m, cn * 512 : (cn + 1) * 512], in_=ot)

    nc.compile()
    return nc


_PERM = np.concatenate([np.arange(0, D, 2), np.arange(1, D, 2)])


def make_in_maps(x, freqs_cos, freqs_sin, Wq, Wk, Wv, Wo):
    x = np.asarray(x, dtype=np.float32)
    freqs_cos = np.asarray(freqs_cos, dtype=np.float32)
    freqs_sin = np.asarray(freqs_sin, dtype=np.float32)
    Wq = np.asarray(Wq, dtype=np.float32)
    Wk = np.asarray(Wk, dtype=np.float32)
    Wv = np.asarray(Wv, dtype=np.float32)
    Wo = np.asarray(Wo, dtype=np.float32)

    scale = 1.0 / np.sqrt(np.float32(D))
    cosT = np.ascontiguousarray(freqs_cos.T)  # [64, T]
    sinT = np.ascontiguousarray(freqs_sin.T)
    cc = np.ascontiguousarray(np.concatenate([cosT, cosT], axis=0))  # [128, T]
    ss = np.ascontiguousarray(np.concatenate([-sinT, sinT], axis=0))
    wk_p = np.ascontiguousarray(Wk[:, _PERM])
    wv_c = np.ascontiguousarray(Wv)

    xts = [np.ascontiguousarray(x[b].T) for b in range(2)]

    ones_a = np.ones((128, 128), dtype=np.float32)
    ident_a = np.eye(128, dtype=np.float32)
    in_maps = []
    for core in range(N_CORES):
        b = core // 4
        hg = core % 4
        heads = range(4 * hg, 4 * hg + 4)
        qcols = np.concatenate([h * D + _PERM for h in heads])
        wq_c = np.ascontiguousarray(Wq[:, qcols] * scale)
        orows = np.concatenate([np.arange(h * D, (h + 1) * D) for h in heads])
        wo_c = np.ascontiguousarray(Wo[orows, :])
        in_maps.append(
            {
                "xt": xts[b],
                "wq": wq_c,
                "wk": wk_p,
                "wv": wv_c,
                "wo": wo_c,
                "cc": cc,
                "ss": ss,
                "ones_d": ones_a,
                "ident_d": ident_a,
            }
        )
    return in_maps


_PROGRAM = None


def get_program():
    global _PROGRAM
    if _PROGRAM is None:
        _PROGRAM = build_program()
    return _PROGRAM


def kernel(x, freqs_cos, freqs_sin, Wq, Wk, Wv, Wo, _collect=None):
    nc = get_program()
    in_maps = make_in_maps(x, freqs_cos, freqs_sin, Wq, Wk, Wv, Wo)
    res = run_bass_kernel_spmd(nc, in_maps, core_ids=list(range(N_CORES)))
    if _collect is not None:
        _collect.append(res)
    outs = [r["out"] for r in res.results]
    full = np.empty((2, T, C), dtype=np.float32)
    for b in range(2):
        full[b] = outs[4 * b] + outs[4 * b + 1] + outs[4 * b + 2] + outs[4 * b + 3]
    return full


# revision 15
# speedup vs baseline: 1.2309x; 1.0115x over previous
"""MQA attention kernel (B=2, T=2048, C=2048, 16 query heads, D=128, RoPE,
causal) for 8 Trainium2 NeuronCores.

Sharding: core = (batch, head-group-of-4). Each core computes q projection for
its 4 heads, the full shared K/V projection for its batch (MQA), causal
attention, and a partial output projection; the host sums the 4 partials per
batch.

Device layout notes:
  - Host pre-transposes x to xT [C, T] so all contractions over C read
    contiguous DRAM.
  - RoPE's even/odd interleave is turned into a half-split layout by permuting
    Wq/Wk columns on the host (scores are invariant to a shared permutation of
    the head dim).  Wq is also pre-scaled by 1/sqrt(D).
  - Scores are computed transposed, S.T[j, i], so the p@V and output
    projections need no on-chip transposes; softmax denominators come from a
    ones-row matmul and are applied via a DRAM-broadcast + reciprocal.
  - All matmuls run as float32r (full PE rate for moving dim >= 256).
"""

import os
import sys

if "/opt/trn_rl_repo" not in sys.path:
    sys.path.insert(0, "/opt/trn_rl_repo")

import numpy as np

import concourse.bacc as bacc
import concourse.mybir as mybir
import concourse.tile as tile
from concourse.bass_utils import run_bass_kernel_spmd

T = 2048
C = 2048
D = 128
N_HEAD = 16
HPC = 4  # heads per core
N_CORES = 8
F32 = mybir.dt.float32
F32R = mybir.dt.float32r
EXP = mybir.ActivationFunctionType.Exp


KPHASE = int(os.environ.get("KPHASE", "9"))
KSUM = int(os.environ.get("KSUM", "1"))
KEXP = int(os.environ.get("KEXP", "1"))


def build_program():
    nc = bacc.Bacc("TRN2", target_bir_lowering=False, debug=False)

    xt = nc.dram_tensor("xt", [C, T], F32R, kind="ExternalInput")
    wq = nc.dram_tensor("wq", [C, HPC * D], F32R, kind="ExternalInput")
    wk = nc.dram_tensor("wk", [C, D], F32R, kind="ExternalInput")
    wv = nc.dram_tensor("wv", [C, D], F32R, kind="ExternalInput")
    wo = nc.dram_tensor("wo", [HPC * D, C], F32R, kind="ExternalInput")
    cc = nc.dram_tensor("cc", [D, T], F32, kind="ExternalInput")
    ss = nc.dram_tensor("ss", [D, T], F32, kind="ExternalInput")
    ones_d = nc.dram_tensor("ones_d", [128, 128], F32R, kind="ExternalInput")
    ident_d = nc.dram_tensor("ident_d", [128, 128], F32R, kind="ExternalInput")
    out = nc.dram_tensor("out", [T, C], F32, kind="ExternalOutput")

    xt_r = xt.rearrange("(ko p) t -> p ko t", p=128)
    wq_r = wq.rearrange("(ko p) m -> p ko m", p=128)
    wk_r = wk.rearrange("(ko p) m -> p ko m", p=128)
    wv_r = wv.rearrange("(ko p) m -> p ko m", p=128)
    wo_r = wo.rearrange("(ho p) c -> p ho c", p=128)
    out_r = out.rearrange("(mo p) c -> p mo c", p=128)

    with (
        tile.TileContext(nc) as tc,
        tc.tile_pool(name="consts", bufs=1) as consts,
        tc.tile_pool(name="qkpool", bufs=20) as qkpool,
        tc.tile_pool(name="ytpool", bufs=16) as ytpool,
        tc.tile_pool(name="wpool", bufs=1) as wpool,
    ):
        ident = consts.tile([128, 128], F32R, tag="ident")
        nc.sync.dma_start(out=ident, in_=ident_d[:, :])
        ones = consts.tile([128, 128], F32R, tag="ones")
        nc.sync.dma_start(out=ones, in_=ones_d[:, :])
        # tri[j, i] = 1 if i >= j else 0 (keep causal-valid entries)
        tri = consts.tile([128, 128], F32, tag="tri")
        nc.gpsimd.memset(tri, 1.0)
        nc.gpsimd.affine_select(
            out=tri,
            in_=tri,
            compare_op=mybir.AluOpType.is_ge,
            fill=0.0,
            base=0,
            pattern=[[1, 128]],
            channel_multiplier=-1,
        )
        ccs = consts.tile([128, T], F32, tag="cc")
        nc.sync.dma_start(out=ccs, in_=cc[:, :])
        sss = consts.tile([128, T], F32, tag="ss")
        nc.sync.dma_start(out=sss, in_=ss[:, :])
        wks = consts.tile([128, 16, 128], F32R, tag="wk")
        nc.sync.dma_start(out=wks, in_=wk_r)
        wvs = consts.tile([128, 16, 128], F32R, tag="wv")
        nc.sync.dma_start(out=wvs, in_=wv_r)
        vsb = consts.tile([128, 16, 128], F32R, tag="vsb")  # v, natural [t, d]

        wqs = wpool.tile([128, 16, 512], F32R, tag="w")
        nc.sync.dma_start(out=wqs, in_=wq_r)

        # qk[idx][c] = 512-wide chunk c of q.T (idx<4) / k.T (idx=4), RoPE'd
        qk = [
            [qkpool.tile([128, 512], F32R, tag="qk", name=f"qk{i}_{c}") for c in range(4)]
            for i in range(5)
        ]
        yt = [
            [ytpool.tile([128, 512], F32R, tag="yt", name=f"yt{i}_{c}") for c in range(4)]
            for i in range(4)
        ]

        # ---- phase 1: q/k/v projections (contraction over C) ----
        with (
            tc.tile_pool(name="t512", bufs=18) as t512,
            tc.tile_pool(name="psA", bufs=4, space="PSUM") as psA,
            tc.tile_pool(name="psY", bufs=2, space="PSUM") as psY,
            tc.tile_pool(name="psS", bufs=2, space="PSUM") as psS,
        ):
            for tcn in range(4):
                tsl = slice(tcn * 512, (tcn + 1) * 512)
                xts = []
                for k in range(16):
                    xtt = t512.tile([128, 512], F32R, tag="t512", name=f"xt{tcn}_{k}")
                    nc.sync.dma_start(out=xtt, in_=xt_r[:, k, tsl])
                    xts.append(xtt)
                # group 1: the 4 q heads
                pq = [psA.tile([128, 512], F32, tag="ps", name=f"pq{i}") for i in range(4)]
                for k in range(16):
                    st, sp = k == 0, k == 15
                    for h in range(4):
                        nc.tensor.matmul(
                            pq[h],
                            wqs[:, k, h * 128 : (h + 1) * 128],
                            xts[k],
                            start=st,
                            stop=sp,
                        )
                # group 2: kT and vT
                pk = psA.tile([128, 512], F32, tag="ps")
                pv = psA.tile([128, 512], F32, tag="ps")
                for k in range(16):
                    st, sp = k == 0, k == 15
                    nc.tensor.matmul(pk, wks[:, k, :], xts[k], start=st, stop=sp)
                    nc.tensor.matmul(pv, wvs[:, k, :], xts[k], start=st, stop=sp)
                for h in range(4):
                    nc.scalar.copy(out=qk[h][tcn], in_=pq[h])
                nc.scalar.copy(out=qk[4][tcn], in_=pk)

                # v natural tiles for this chunk via PE transpose
                vtt = t512.tile([128, 512], F32R, tag="t512", name=f"vtt{tcn}")
                nc.scalar.copy(out=vtt, in_=pv)
                for mm in range(4):
                    m = tcn * 4 + mm
                    ptp = psA.tile([128, 512], F32R, tag="ps", name=f"ptp{m}")
                    nc.tensor.transpose(
                        ptp[:, :128], vtt[:, mm * 128 : (mm + 1) * 128], ident
                    )
                    nc.vector.tensor_copy(out=vsb[:, m, :], in_=ptp[:, :128])

                # RoPE this chunk (k first so attention unblocks earliest)
                for idx in [4, 0, 1, 2, 3]:
                    qc = qk[idx][tcn]
                    sw = t512.tile([128, 512], F32R, tag="t512", name=f"sw{tcn}_{idx}")
                    nc.gpsimd.dma_start(out=sw[0:64, :], in_=qc[64:128, :])
                    nc.gpsimd.dma_start(out=sw[64:128, :], in_=qc[0:64, :])
                    nc.vector.tensor_mul(out=qc[:], in0=qc[:], in1=ccs[:, tsl])
                    nc.vector.tensor_mul(out=sw[:], in0=sw[:], in1=sss[:, tsl])
                    nc.vector.tensor_add(out=qc[:], in0=qc[:], in1=sw[:])

            # ---- phase 2: causal attention, scores transposed S.T[j, i] ----
            for h in range(4 if KPHASE >= 2 else 0):
                for c in range(4):
                    i0 = c * 512
                    py = psY.tile([128, 512], F32, tag="py")
                    psm = psS.tile([128, 512], F32, tag="pss")
                    njj = 4 * c + 4
                    for jj in range(njj):
                        r = jj - 4 * c  # >= 0 only for diagonal-group tiles
                        off = 128 * r if r >= 0 else 0
                        pss = psA.tile([128, 512], F32, tag="ps")
                        nc.tensor.matmul(
                            pss[:, off:],
                            qk[4][jj // 4][:, (jj % 4) * 128 : (jj % 4 + 1) * 128],
                            qk[h][c][:, off:],
                            start=True,
                            stop=True,
                        )
                        pT = t512.tile([128, 512], F32R, tag="t512")
                        if KEXP:
                            nc.scalar.activation(out=pT[:, off:], in_=pss[:, off:], func=EXP)
                        else:
                            nc.scalar.copy(out=pT[:, off:], in_=pss[:, off:])
                        if r >= 0:
                            nc.vector.tensor_mul(
                                out=pT[:, off : off + 128],
                                in0=pT[:, off : off + 128],
                                in1=tri,
                            )
                        st, sp = jj == 0, jj == njj - 1
                        nc.tensor.matmul(
                            py[:, off:],
                            (vsb[:, jj, :]),
                            (pT[:, off:]),
                            start=st,
                            stop=sp,
                        )
                        nc.tensor.matmul(
                            psm[:, off:],
                            (ones),
                            (pT[:, off:]),
                            start=st,
                            stop=sp,
                        )
                    bc = t512.tile([128, 512], F32, tag="t512")
                    nc.vector.reciprocal_approx_fast(out=bc, in_=psm)
                    nc.vector.tensor_mul(out=yt[h][c], in0=py, in1=bc)

            # ---- phase 3: partial output projection (contraction over d) ----
            wos = wpool.tile([128, 4, T], F32R, tag="w")
            nc.gpsimd.dma_start(out=wos, in_=wo_r)
            for m in range(16 if KPHASE >= 4 else 0):
                for cn in range(4):
                    po = psA.tile([128, 512], F32, tag="ps")
                    for h in range(4):
                        nc.tensor.matmul(
                            po,
                            yt[h][m // 4][:, (m % 4) * 128 : (m % 4 + 1) * 128],
                            wos[:, h, cn * 512 : (cn + 1) * 512],
                            start=h == 0,
                            stop=h == 3,
                        )
                    ot = t512.tile([128, 512], F32, tag="t512")
                    nc.any.tensor_copy(out=ot, in_=po)
                    nc.sync.dma_start(out=out_r[:, m, cn * 512 : (cn + 1) * 512], in_=ot)

    nc.compile()
    return nc


_PERM = np.concatenate([np.arange(0, D, 2), np.arange(1, D, 2)])


def make_in_maps(x, freqs_cos, freqs_sin, Wq, Wk, Wv, Wo):
    x = np.asarray(x, dtype=np.float32)
    freqs_cos = np.asarray(freqs_cos, dtype=np.float32)
    freqs_sin = np.asarray(freqs_sin, dtype=np.float32)
    Wq = np.asarray(Wq, dtype=np.float32)
    Wk = np.asarray(Wk, dtype=np.float32)
    Wv = np.asarray(Wv, dtype=np.float32)
    Wo = np.asarray(Wo, dtype=np.float32)

    scale = 1.0 / np.sqrt(np.float32(D))
    cosT = np.ascontiguousarray(freqs_cos.T)  # [64, T]
    sinT = np.ascontiguousarray(freqs_sin.T)
    cc = np.ascontiguousarray(np.concatenate([cosT, cosT], axis=0))  # [128, T]
    ss = np.ascontiguousarray(np.concatenate([-sinT, sinT], axis=0))
    wk_p = np.ascontiguousarray(Wk[:, _PERM])
    wv_c = np.ascontiguousarray(Wv)

    xts = [np.ascontiguousarray(x[b].T) for b in range(2)]

    ones_a = np.ones((128, 128), dtype=np.float32)
    ident_a = np.eye(128, dtype=np.float32)
    in_maps = []
    for core in range(N_CORES):
        b = core // 4
        hg = core % 4
        heads = range(4 * hg, 4 * hg + 4)
        qcols = np.concatenate([h * D + _PERM for h in heads])
        wq_c = np.ascontiguousarray(Wq[:, qcols] * scale)
        orows = np.concatenate([np.arange(h * D, (h + 1) * D) for h in heads])
        wo_c = np.ascontiguousarray(Wo[orows, :])
        in_maps.append(
            {
                "xt": xts[b],
                "wq": wq_c,
                "wk": wk_p,
                "wv": wv_c,
                "wo": wo_c,
                "cc": cc,
                "ss": ss,
                "ones_d": ones_a,
                "ident_d": ident_a,
            }
        )
    return in_maps


_PROGRAM = None


def get_program():
    global _PROGRAM
    if _PROGRAM is None:
        _PROGRAM = build_program()
    return _PROGRAM


def kernel(x, freqs_cos, freqs_sin, Wq, Wk, Wv, Wo, _collect=None):
    nc = get_program()
    in_maps = make_in_maps(x, freqs_cos, freqs_sin, Wq, Wk, Wv, Wo)
    res = run_bass_kernel_spmd(nc, in_maps, core_ids=list(range(N_CORES)))
    if _collect is not None:
        _collect.append(res)
    outs = [r["out"] for r in res.results]
    full = np.empty((2, T, C), dtype=np.float32)
    for b in range(2):
        full[b] = outs[4 * b] + outs[4 * b + 1] + outs[4 * b + 2] + outs[4 * b + 3]
    return full
